# revision 8
# baseline (speedup 1.0000x reference)
"""Trainium2 Bass kernel for multi-head self-attention (B=8, N=1024, C=768, H=12).

Sharding: data-parallel over batch -- one batch element per NeuronCore (8 cores).

Key design points (v2, rewritten from the f32r baseline):
  * All operand transposes are done on the HOST (numpy) -- the device kernel
    contains ZERO PE-transpose instructions.  PE transposes don't count as
    PE-busy for the HAM clock gate, so the baseline's 288 transposes both
    cost ~80us of PE slices and kept re-throttling the PE to 1.2 GHz.
  * Everything is bf16 (1.0 cycles/row, FWL-eligible weight loads, half DMA).
  * v is computed in NATURAL [n, c] layout via an xT-stationary GEMM
    (out[n,vc] = xT[k,n].T @ wvT[k,vc]) so the per-head [keys, 64|1] PV
    stationary tiles are built with cheap DVE copies instead of PE transposes.
  * ST score matmuls have K=64: the two heads of a pair sit on SBUF
    partitions 0:64 / 64:128, so their matmuls land on disjoint PE row
    groups (tile_position (0,0) vs (64,0)) and execute CONCURRENTLY when
    issued back-to-back (~2x effective ST throughput).
  * Per head pair the PE work (qk GEMM for next pair + ST + PV + norm
    broadcasts ~16.2us) matches the ACT exp work (16 x [128,1024] exp =
    16.4us), so both engines stay ~100% busy and the HAM stays at K=8/8.

Per-core dataflow:
  qkT  [256, N] per pair = wqkT_pair.T @ xT   (bf16, stationary = w slices)
  v    [N, C]  = xT.T @ wvT                    (bf16, natural layout)
  per head h:  ST[m,n] = k_h @ q_h^T           (bf16, K=64, row-tiled pairs)
               ET = exp(0.125*ST) -> bf16      (one ACT op per [128,1024])
               PV: [v_h | 1].T @ ET -> [65, n] unnormalized + denominator
               OT[d,n] = PV[0:64] * bcast(den)^-1   (K=1 PE bcast + DVE)
  out [N, C] = OT.T @ wprojT + b_proj          (bf16, bias as K=1 f32r matmul)
"""

import numpy as np
import ml_dtypes

import concourse.bass as bass
import concourse.tile as tile
from concourse import bacc
from concourse import mybir
from concourse.bass_utils import run_bass_kernel_spmd

N = 1024
C = 768
H = 12
D = 64
NCORES = 8
SCALE = D**-0.5

F32 = mybir.dt.float32
F32R = mybir.dt.float32r
BF16 = mybir.dt.bfloat16
EXP = mybir.ActivationFunctionType.Exp

NT_N = N // 128  # 8 n-blocks / key tiles
NT_C = C // 128  # 6 k-chunks
NPAIR = H // 2   # 6 head pairs

BF = ml_dtypes.bfloat16


def build_bass():
    nc = bacc.Bacc("TRN2", target_bir_lowering=False, debug=False, num_devices=NCORES)

    # host-pretransposed inputs
    xT_d = nc.dram_tensor("xT", [C, N], BF16, kind="ExternalInput").ap()
    # per-pair packed [q_pair(128) | k_pair(128)] columns: [NPAIR*768, 256]
    wqk_d = nc.dram_tensor("wqk", [NPAIR * C, 256], BF16, kind="ExternalInput").ap()
    wv_d = nc.dram_tensor("wv", [C, C], BF16, kind="ExternalInput").ap()  # w_v^T
    wp_d = nc.dram_tensor("wp", [C, C], BF16, kind="ExternalInput").ap()  # w_proj^T
    b_d = nc.dram_tensor("b_proj", [1, C], F32R, kind="ExternalInput").ap()
    out_d = nc.dram_tensor("out", [N, C], F32, kind="ExternalOutput").ap()

    with tile.TileContext(nc) as tc:
        with (
            tc.tile_pool(name="singles", bufs=1) as singles,
            tc.tile_pool(name="wqk", bufs=2) as p_wqk,
            tc.tile_pool(name="qk", bufs=2) as p_qk,
            tc.tile_pool(name="et", bufs=32) as p_et,
            tc.tile_pool(name="OT", bufs=NPAIR) as p_OT,
            tc.tile_pool(name="pvs", bufs=4) as p_pvs,
            tc.tile_pool(name="rcb", bufs=4) as p_rcb,
            tc.tile_pool(name="ott", bufs=2) as p_ott,
            tc.tile_pool(name="osb", bufs=2) as p_osb,
            # PSUM: 8 banks total
            tc.tile_pool(name="pp_st", bufs=2, space="PSUM") as pp_st,  # 2x2 banks
            tc.tile_pool(name="pp_sm", bufs=4, space="PSUM") as pp_sm,  # 4x1 bank
        ):
            # ---------------- setup ----------------
            ones_f = singles.tile([128, 128], F32, tag="ones_f")
            nc.vector.memset(ones_f[:], 1.0)
            ones_r = singles.tile([128, 128], F32R, tag="ones_r")
            nc.vector.tensor_copy(ones_r[:], ones_f[:])
            ones_b = singles.tile([128, 1], BF16, tag="ones_b")
            nc.vector.tensor_copy(ones_b[:], ones_f[:, 0:1])
            b_row = singles.tile([1, C], F32R, tag="b_row")
            nc.sync.dma_start(b_row[:], b_d)

            # persistent SBUF planes
            xT = singles.tile([128, NT_C * N], BF16, tag="xT")      # [k, n] chunks
            wv = singles.tile([128, NT_C * C], BF16, tag="wv")      # [k, vc] chunks
            wp = singles.tile([128, NT_C * C], BF16, tag="wp")      # [cj, oc] chunks
            vnat = singles.tile([128, NT_N * C], BF16, tag="vnat")  # [n, vc] blocks
            # per (h, t) PV stationary slots [keys, v(64) | ones]
            vn = singles.tile([128, H * NT_N * 65], BF16, tag="vn")

            # ones column of every vn slot
            ones_cols = bass.AP(
                tensor=vn.tensor,
                offset=vn.offset + 64,
                ap=[vn.ap[0], [65, H * NT_N], [1, 1]],
            )
            ones_rep = bass.AP(
                tensor=ones_b.tensor,
                offset=ones_b.offset,
                ap=[ones_b.ap[0], [0, H * NT_N], [1, 1]],
            )
            nc.vector.tensor_copy(ones_cols, ones_rep)

            # ---------------- input DMAs ----------------
            def dma_wqk(hp):
                t = p_wqk.tile([128, NT_C * 256], BF16, tag="wqk", name=f"wqk{hp}")
                for kc in range(NT_C):
                    nc.sync.dma_start(
                        t[:, kc * 256 : (kc + 1) * 256],
                        wqk_d[hp * C + kc * 128 : hp * C + (kc + 1) * 128, :],
                    )
                return t

            # interleave wqk0/xT chunk loads so the pair-0 qk GEMM can start
            # on chunk 0 ~1.2us in
            wqk_cur = p_wqk.tile([128, NT_C * 256], BF16, tag="wqk", name="wqk0")
            for kc in range(NT_C):
                nc.sync.dma_start(
                    wqk_cur[:, kc * 256 : (kc + 1) * 256],
                    wqk_d[kc * 128 : (kc + 1) * 128, :],
                )
                nc.sync.dma_start(
                    xT[:, kc * N : (kc + 1) * N], xT_d[kc * 128 : (kc + 1) * 128, :]
                )
            for kc in range(NT_C):
                nc.sync.dma_start(
                    wv[:, kc * C : (kc + 1) * C], wv_d[kc * 128 : (kc + 1) * 128, :]
                )
            for kc in range(NT_C):
                nc.sync.dma_start(
                    wp[:, kc * C : (kc + 1) * C], wp_d[kc * 128 : (kc + 1) * 128, :]
                )

            # ---------------- emission helpers ----------------
            def emit_qk_first(wqk_t):
                """Pair-0 qk GEMM, k-chunk outer so PE starts as DMA lands."""
                qk_sb = p_qk.tile([128, 2 * N], BF16, tag="qk", name="qk0")
                ps = {}
                for part in range(2):
                    for nj in range(2):
                        ps[part, nj] = pp_sm.tile(
                            [128, 512], F32, tag="pp_sm", name=f"qkps{part}{nj}"
                        )
                for kc in range(NT_C):
                    for part in range(2):
                        for nj in range(2):
                            nc.tensor.matmul(
                                ps[part, nj][:],
                                wqk_t[:, kc * 256 + part * 128 : kc * 256 + (part + 1) * 128],
                                xT[:, kc * N + nj * 512 : kc * N + nj * 512 + 512],
                                start=(kc == 0),
                                stop=(kc == NT_C - 1),
                            )
                for part in range(2):
                    for nj in range(2):
                        nc.vector.tensor_copy(
                            qk_sb[:, part * N + nj * 512 : part * N + nj * 512 + 512],
                            ps[part, nj][:],
                        )
                return qk_sb

            def emit_qk_group(qk_sb, wqk_t, part, nj):
                """One (part, nj) quarter of a pair's qk GEMM: 6 MMs + copy."""
                ps = pp_sm.tile([128, 512], F32, tag="pp_sm")
                for kc in range(NT_C):
                    nc.tensor.matmul(
                        ps[:],
                        wqk_t[:, kc * 256 + part * 128 : kc * 256 + (part + 1) * 128],
                        xT[:, kc * N + nj * 512 : kc * N + nj * 512 + 512],
                        start=(kc == 0),
                        stop=(kc == NT_C - 1),
                    )
                nc.vector.tensor_copy(
                    qk_sb[:, part * N + nj * 512 : part * N + nj * 512 + 512], ps[:]
                )

            def emit_v_group(nb):
                """v GEMM for one n-block: v_nat[nb] = xT[:, nb].T @ wvT."""
                ps1 = pp_sm.tile([128, 512], F32, tag="pp_sm")
                ps2 = pp_sm.tile([128, 256], F32, tag="pp_sm")
                for kc in range(NT_C):
                    lhsT = xT[:, kc * N + nb * 128 : kc * N + (nb + 1) * 128]
                    nc.tensor.matmul(
                        ps1[:], lhsT, wv[:, kc * C : kc * C + 512],
                        start=(kc == 0), stop=(kc == NT_C - 1),
                    )
                    nc.tensor.matmul(
                        ps2[:], lhsT, wv[:, kc * C + 512 : kc * C + 768],
                        start=(kc == 0), stop=(kc == NT_C - 1),
                    )
                nc.vector.tensor_copy(vnat[:, nb * C : nb * C + 512], ps1[:])
                nc.vector.tensor_copy(vnat[:, nb * C + 512 : nb * C + 768], ps2[:])

            def emit_vn_copies(h, t):
                """Fill vn slot (h, t) from v_nat block t (ones col pre-set)."""
                s = (h * NT_N + t) * 65
                nc.vector.tensor_copy(
                    vn[:, s : s + 64], vnat[:, t * C + h * 64 : t * C + (h + 1) * 64]
                )

            def emit_st(qk_sb, t, ets):
                """Row-tiled concurrent ST pair for heads h0 (rows 0:64) and h1."""
                pss = []
                for h2 in range(2):
                    ps = pp_st.tile([128, N], F32, tag="pp_st")
                    pss.append(ps)
                for nj in range(2):
                    nsl = slice(nj * 512, (nj + 1) * 512)
                    for h2 in range(2):
                        rsl = slice(h2 * 64, h2 * 64 + 64)
                        nc.tensor.matmul(
                            pss[h2][:, nsl],
                            qk_sb[rsl, N + t * 128 : N + (t + 1) * 128],
                            qk_sb[rsl, nsl],
                            start=True,
                            stop=True,
                        )
                for h2 in range(2):
                    e = p_et.tile([128, N], BF16, tag="et")
                    nc.scalar.activation(e[:], pss[h2][:], EXP, scale=SCALE)
                    ets[h2].append(e)

            def emit_pv(h, nj, ets, po):
                """PV for one (head, n-half): accumulate 8 key tiles, M=65."""
                p_ = pp_sm.tile([65, 512], F32, tag="pp_sm")
                po.append(p_)
                nsl = slice(nj * 512, (nj + 1) * 512)
                for t in range(NT_N):
                    s = (h * NT_N + t) * 65
                    nc.tensor.matmul(
                        p_[:],
                        vn[:, s : s + 65],
                        ets[t][:, nsl],
                        start=(t == 0),
                        stop=(t == NT_N - 1),
                    )

            def emit_pv_copy(po, pvs):
                """Move PV psum (unnorm + den row) to SBUF, freeing the bank."""
                for nj in range(2):
                    pv = p_pvs.tile([65, 512], F32R, tag="pvs")
                    nc.vector.tensor_copy(pv[:], po[nj][:])
                    pvs.append(pv)

            def emit_bcast(pvs, pbs):
                """K=1 PE matmul broadcasting den across 64 partitions."""
                for nj in range(2):
                    pb = pp_sm.tile([64, 512], F32, tag="pp_sm")
                    nc.tensor.matmul(
                        pb[:], ones_r[64:65, 0:64], pvs[nj][64:65, :],
                        start=True, stop=True,
                    )
                    pbs.append(pb)

            def emit_norm(h2, OT_hp, ot_tmp, pvs, pbs):
                for nj in range(2):
                    nsl = slice(nj * 512, (nj + 1) * 512)
                    rcb = p_rcb.tile([64, 512], F32, tag="rcb")
                    nc.vector.reciprocal_approx_fast(rcb[:], pbs[nj][:])
                    dst = OT_hp[0:64, nsl] if h2 == 0 else ot_tmp[:, nsl]
                    nc.vector.tensor_mul(dst, pvs[nj][0:64, :], rcb[:])

            # ---------------- PE warm-up ----------------
            # ~8 x 512-row dummy matmuls keep the PE busy through the HAM
            # SHORT window while the first DMA chunks land, so the pair-0
            # qk GEMM runs at 2.4 GHz instead of 1.2.
            ones_bb = singles.tile([128, 128], BF16, tag="ones_bb")
            nc.vector.tensor_copy(ones_bb[:], ones_f[:])
            warm_src = singles.tile([128, 512], BF16, tag="warm_src")
            nc.vector.memset(warm_src[:], 0.0)
            ps_warm = pp_sm.tile([128, 512], F32, tag="pp_sm", name="ps_warm")
            for _ in range(8):
                nc.tensor.matmul(
                    ps_warm[:], ones_bb[:], warm_src[:], start=True, stop=True
                )

            # ---------------- pair 0 qk GEMM (DMA-pipelined) ----------------
            qk_cur = emit_qk_first(wqk_cur)

            # ---------------- steady-state pair loop ----------------
            ets_prev = None   # [h2] -> list of 8 et tiles, previous pair
            hp_prev = None
            OT = [
                p_OT.tile([128, N], BF16, tag="OT", name=f"OT{j}")
                for j in range(NPAIR)
            ]

            for hp in range(NPAIR):
                if hp + 1 < NPAIR:
                    wqk_next = dma_wqk(hp + 1)
                    qk_next = p_qk.tile([128, 2 * N], BF16, tag="qk", name=f"qk{hp+1}")
                ets = [[], []]
                # filler state for PV(hp-1) + norm
                po_h = [[], []]
                pvs_h = [[], []]
                pbs_h = [[], []]
                if hp_prev is not None:
                    OT_prev = OT[hp_prev]
                    ot_tmp = p_ott.tile([64, N], BF16, tag="ott")

                for t in range(NT_N):
                    emit_st(qk_cur, t, ets)
                    if hp == 0:
                        # pair 0 fillers: v GEMM + vn assembly + qk GEMM pair 1
                        emit_v_group(t)
                        for h in (0, 1):  # vn slots for pair 0's heads
                            emit_vn_copies(h, t)
                        if t in (2, 3, 6, 7):
                            part, nj = {2: (0, 0), 3: (0, 1), 6: (1, 0), 7: (1, 1)}[t]
                            emit_qk_group(qk_next, wqk_next, part, nj)
                    else:
                        h_lo = 2 * hp_prev
                        if t == 0:
                            emit_pv(h_lo, 0, ets_prev[0], po_h[0])
                        elif t == 1:
                            emit_pv(h_lo, 1, ets_prev[0], po_h[0])
                        elif t == 2:
                            emit_pv_copy(po_h[0], pvs_h[0])
                            emit_bcast(pvs_h[0], pbs_h[0])
                            if hp + 1 < NPAIR:
                                emit_qk_group(qk_next, wqk_next, 0, 0)
                        elif t == 3:
                            emit_norm(0, OT_prev, None, pvs_h[0], pbs_h[0])
                            if hp + 1 < NPAIR:
                                emit_qk_group(qk_next, wqk_next, 0, 1)
                        elif t == 4:
                            emit_pv(h_lo + 1, 0, ets_prev[1], po_h[1])
                        elif t == 5:
                            emit_pv(h_lo + 1, 1, ets_prev[1], po_h[1])
                        elif t == 6:
                            emit_pv_copy(po_h[1], pvs_h[1])
                            emit_bcast(pvs_h[1], pbs_h[1])
                            if hp + 1 < NPAIR:
                                emit_qk_group(qk_next, wqk_next, 1, 0)
                        elif t == 7:
                            emit_norm(1, OT_prev, ot_tmp, pvs_h[1], pbs_h[1])
                            nc.sync.dma_start(OT_prev[64:128, :], ot_tmp[:])
                            if hp + 1 < NPAIR:
                                emit_qk_group(qk_next, wqk_next, 1, 1)
                        # vn slots for this pair's heads (needed by PV at hp+1)
                        for h in (2 * hp, 2 * hp + 1):
                            emit_vn_copies(h, t)

                ets_prev = ets
                hp_prev = hp
                if hp + 1 < NPAIR:
                    qk_cur = qk_next
                    wqk_cur = wqk_next

            # ---------------- tail: PV + norm of last pair, interleaved with
            # proj partials for nb 0/1 so the PE never idles through the
            # final exp drain (idle >3.4us would re-throttle the HAM).
            def emit_proj_partial(ps, nb, jmax):
                for osl in (slice(0, 512), slice(512, 768)):
                    for j in range(jmax):
                        nc.tensor.matmul(
                            ps[:, osl],
                            OT[j][:, nb * 128 : (nb + 1) * 128],
                            wp[:, j * C + osl.start : j * C + osl.stop],
                            start=(j == 0),
                            stop=False,
                        )

            def emit_proj_finish(ps, nb, jmin):
                for osl in (slice(0, 512), slice(512, 768)):
                    for j in range(jmin, NPAIR):
                        nc.tensor.matmul(
                            ps[:, osl],
                            OT[j][:, nb * 128 : (nb + 1) * 128],
                            wp[:, j * C + osl.start : j * C + osl.stop],
                            start=(jmin == 0 and j == 0),
                            stop=False,
                        )
                    nc.tensor.matmul(
                        ps[:, osl],
                        ones_r[0:1, 0:128],
                        b_row[:, osl],
                        start=False,
                        stop=True,
                    )
                osb = p_osb.tile([128, C], F32, tag="osb", name=f"osb{nb}")
                nc.vector.tensor_copy(osb[:], ps[:, 0:C])
                nc.sync.dma_start(out_d[nb * 128 : (nb + 1) * 128, :], osb[:])

            h_lo = 2 * hp_prev
            OT_prev = OT[hp_prev]
            ot_tmp = p_ott.tile([64, N], BF16, tag="ott")
            po_h = [[], []]
            pvs_h = [[], []]
            pbs_h = [[], []]

            ps_nb0 = pp_st.tile([128, N], F32, tag="pp_st", name="ps_nb0")
            emit_proj_partial(ps_nb0, 0, NPAIR - 1)
            emit_pv(h_lo, 0, ets_prev[0], po_h[0])
            emit_pv(h_lo, 1, ets_prev[0], po_h[0])
            ps_nb1 = pp_st.tile([128, N], F32, tag="pp_st", name="ps_nb1")
            emit_proj_partial(ps_nb1, 1, NPAIR - 1)
            emit_pv(h_lo + 1, 0, ets_prev[1], po_h[1])
            emit_pv_copy(po_h[0], pvs_h[0])
            emit_pv(h_lo + 1, 1, ets_prev[1], po_h[1])
            emit_bcast(pvs_h[0], pbs_h[0])
            emit_norm(0, OT_prev, None, pvs_h[0], pbs_h[0])
            emit_pv_copy(po_h[1], pvs_h[1])
            emit_bcast(pvs_h[1], pbs_h[1])
            emit_norm(1, OT_prev, ot_tmp, pvs_h[1], pbs_h[1])
            nc.sync.dma_start(OT_prev[64:128, :], ot_tmp[:])

            # ---------------- proj ----------------
            emit_proj_finish(ps_nb0, 0, NPAIR - 1)
            emit_proj_finish(ps_nb1, 1, NPAIR - 1)
            for nb in range(2, NT_N):
                ps = pp_st.tile([128, N], F32, tag="pp_st", name=f"ps_nb{nb}")
                emit_proj_finish(ps, nb, 0)

    nc.compile()
    return nc


_NC_CACHE = None


def _prep_inputs(x, w_qkv, w_proj, b_proj):
    x = np.asarray(x, dtype=np.float32)
    w_qkv = np.asarray(w_qkv, dtype=np.float32)
    w_proj = np.asarray(w_proj, dtype=np.float32)
    b_row = np.ascontiguousarray(
        np.asarray(b_proj, dtype=np.float32).reshape(1, C)
    )

    # per-pair packed [768, 256] blocks: cols 0:128 = q rows of the pair
    # transposed, cols 128:256 = k rows of the pair transposed
    wqk_blocks = []
    for hp in range(NPAIR):
        qb = w_qkv[hp * 128 : (hp + 1) * 128, :]          # [128, 768]
        kb = w_qkv[C + hp * 128 : C + (hp + 1) * 128, :]  # [128, 768]
        wqk_blocks.append(np.concatenate([qb.T, kb.T], axis=1))  # [768, 256]
    wqk = np.ascontiguousarray(np.concatenate(wqk_blocks, axis=0)).astype(BF)
    wv = np.ascontiguousarray(w_qkv[2 * C :, :].T).astype(BF)   # [768, 768]
    wp = np.ascontiguousarray(w_proj.T).astype(BF)              # [768, 768]
    xTs = [np.ascontiguousarray(x[b].T).astype(BF) for b in range(NCORES)]
    return xTs, wqk, wv, wp, b_row


def kernel(x, w_qkv, w_proj, b_proj):
    global _NC_CACHE
    if _NC_CACHE is None:
        _NC_CACHE = build_bass()
    nc = _NC_CACHE

    xTs, wqk, wv, wp, b_row = _prep_inputs(x, w_qkv, w_proj, b_proj)
    in_maps = [
        {"xT": xTs[b], "wqk": wqk, "wv": wv, "wp": wp, "b_proj": b_row}
        for b in range(NCORES)
    ]
    res = run_bass_kernel_spmd(nc, in_maps, list(range(NCORES)))
    return np.stack([res.results[b]["out"] for b in range(NCORES)], axis=0)


# revision 18
# speedup vs baseline: 1.0724x; 1.0724x over previous
"""Trainium2 Bass kernel for multi-head self-attention (B=8, N=1024, C=768, H=12).

Sharding: data-parallel over batch -- one batch element per NeuronCore (8 cores).

Key design points (v2, rewritten from the f32r baseline):
  * All operand transposes are done on the HOST (numpy) -- the device kernel
    contains ZERO PE-transpose instructions.  PE transposes don't count as
    PE-busy for the HAM clock gate, so the baseline's 288 transposes both
    cost ~80us of PE slices and kept re-throttling the PE to 1.2 GHz.
  * Everything is bf16 (1.0 cycles/row, FWL-eligible weight loads, half DMA).
  * v is computed in NATURAL [n, c] layout via an xT-stationary GEMM
    (out[n,vc] = xT[k,n].T @ wvT[k,vc]) so the per-head [keys, 64|1] PV
    stationary tiles are built with cheap DVE copies instead of PE transposes.
  * ST score matmuls have K=64: the two heads of a pair sit on SBUF
    partitions 0:64 / 64:128, so their matmuls land on disjoint PE row
    groups (tile_position (0,0) vs (64,0)) and execute CONCURRENTLY when
    issued back-to-back (~2x effective ST throughput).
  * Per head pair the PE work (qk GEMM for next pair + ST + PV + norm
    broadcasts ~16.2us) matches the ACT exp work (16 x [128,1024] exp =
    16.4us), so both engines stay ~100% busy and the HAM stays at K=8/8.

Per-core dataflow:
  qkT  [256, N] per pair = wqkT_pair.T @ xT   (bf16, stationary = w slices)
  v    [N, C]  = xT.T @ wvT                    (bf16, natural layout)
  per head h:  ST[m,n] = k_h @ q_h^T           (bf16, K=64, row-tiled pairs)
               ET = exp(0.125*ST) -> bf16      (one ACT op per [128,1024])
               PV: [v_h | 1].T @ ET -> [65, n] unnormalized + denominator
               OT[d,n] = PV[0:64] * bcast(den)^-1   (K=1 PE bcast + DVE)
  out [N, C] = OT.T @ wprojT + b_proj          (bf16, bias as K=1 f32r matmul)
"""

import numpy as np
import ml_dtypes

import concourse.bass as bass
import concourse.tile as tile
from concourse import bacc
from concourse import mybir
from concourse.bass_utils import run_bass_kernel_spmd

N = 1024
C = 768
H = 12
D = 64
NCORES = 8
SCALE = D**-0.5

F32 = mybir.dt.float32
F32R = mybir.dt.float32r
BF16 = mybir.dt.bfloat16
EXP = mybir.ActivationFunctionType.Exp

NT_N = N // 128  # 8 n-blocks / key tiles
NT_C = C // 128  # 6 k-chunks
NPAIR = H // 2   # 6 head pairs

BF = ml_dtypes.bfloat16


def build_bass():
    nc = bacc.Bacc("TRN2", target_bir_lowering=False, debug=False, num_devices=NCORES)

    # host-pretransposed inputs
    xT_d = nc.dram_tensor("xT", [C, N], BF16, kind="ExternalInput").ap()
    # per-pair packed [q_pair(128) | k_pair(128)] columns: [NPAIR*768, 256]
    wqk_d = nc.dram_tensor("wqk", [NPAIR * C, 256], BF16, kind="ExternalInput").ap()
    wv_d = nc.dram_tensor("wv", [C, C], BF16, kind="ExternalInput").ap()  # w_v^T
    wp_d = nc.dram_tensor("wp", [C, C], BF16, kind="ExternalInput").ap()  # w_proj^T
    b_d = nc.dram_tensor("b_proj", [1, C], F32R, kind="ExternalInput").ap()
    out_d = nc.dram_tensor("out", [N, C], F32, kind="ExternalOutput").ap()

    with tile.TileContext(nc) as tc:
        with (
            tc.tile_pool(name="singles", bufs=1) as singles,
            tc.tile_pool(name="wqk", bufs=2) as p_wqk,
            tc.tile_pool(name="qk", bufs=2) as p_qk,
            tc.tile_pool(name="et", bufs=32) as p_et,
            tc.tile_pool(name="OT", bufs=NPAIR) as p_OT,
            tc.tile_pool(name="pvs", bufs=4) as p_pvs,
            tc.tile_pool(name="rcb", bufs=4) as p_rcb,
            tc.tile_pool(name="osb", bufs=2) as p_osb,
            # PSUM: 8 banks total
            tc.tile_pool(name="pp_st", bufs=2, space="PSUM") as pp_st,  # 2x2 banks
            tc.tile_pool(name="pp_sm", bufs=4, space="PSUM") as pp_sm,  # 4x1 bank
        ):
            # ---------------- setup ----------------
            ones_f = singles.tile([128, 128], F32, tag="ones_f")
            nc.vector.memset(ones_f[:], 1.0)
            ones_r = singles.tile([128, 128], F32R, tag="ones_r")
            nc.vector.tensor_copy(ones_r[:], ones_f[:])
            ones_b = singles.tile([128, 1], BF16, tag="ones_b")
            nc.vector.tensor_copy(ones_b[:], ones_f[:, 0:1])
            b_row = singles.tile([1, C], F32R, tag="b_row")
            nc.sync.dma_start(b_row[:], b_d)

            # persistent SBUF planes
            xT = singles.tile([128, NT_C * N], BF16, tag="xT")      # [k, n] chunks
            wv = singles.tile([128, NT_C * C], BF16, tag="wv")      # [k, vc] chunks
            wp = singles.tile([128, NT_C * C], BF16, tag="wp")      # [cj, oc] chunks
            vnat = singles.tile([128, NT_N * C], BF16, tag="vnat")  # [n, vc] blocks
            # per (h, t) PV stationary slots [keys, v(64) | ones]
            vn = singles.tile([128, H * NT_N * 65], BF16, tag="vn")

            # ones column of every vn slot
            ones_cols = bass.AP(
                tensor=vn.tensor,
                offset=vn.offset + 64,
                ap=[vn.ap[0], [65, H * NT_N], [1, 1]],
            )
            ones_rep = bass.AP(
                tensor=ones_b.tensor,
                offset=ones_b.offset,
                ap=[ones_b.ap[0], [0, H * NT_N], [1, 1]],
            )
            nc.vector.tensor_copy(ones_cols, ones_rep)

            # ---------------- input DMAs ----------------
            def dma_wqk(hp):
                t = p_wqk.tile([128, NT_C * 256], BF16, tag="wqk", name=f"wqk{hp}")
                for kc in range(NT_C):
                    nc.sync.dma_start(
                        t[:, kc * 256 : (kc + 1) * 256],
                        wqk_d[hp * C + kc * 128 : hp * C + (kc + 1) * 128, :],
                    )
                return t

            # interleave wqk0/xT chunk loads so the pair-0 qk GEMM can start
            # on chunk 0 ~1.2us in
            wqk_cur = p_wqk.tile([128, NT_C * 256], BF16, tag="wqk", name="wqk0")
            for kc in range(NT_C):
                nc.sync.dma_start(
                    wqk_cur[:, kc * 256 : (kc + 1) * 256],
                    wqk_d[kc * 128 : (kc + 1) * 128, :],
                )
                nc.sync.dma_start(
                    xT[:, kc * N : (kc + 1) * N], xT_d[kc * 128 : (kc + 1) * 128, :]
                )
            for kc in range(NT_C):
                nc.sync.dma_start(
                    wv[:, kc * C : (kc + 1) * C], wv_d[kc * 128 : (kc + 1) * 128, :]
                )
            for kc in range(NT_C):
                nc.sync.dma_start(
                    wp[:, kc * C : (kc + 1) * C], wp_d[kc * 128 : (kc + 1) * 128, :]
                )

            # ---------------- emission helpers ----------------
            def emit_qk_first(wqk_t):
                """Pair-0 qk GEMM, k-chunk outer so PE starts as DMA lands."""
                qk_sb = p_qk.tile([128, 2 * N], BF16, tag="qk", name="qk0")
                ps = {}
                for part in range(2):
                    for nj in range(2):
                        ps[part, nj] = pp_sm.tile(
                            [128, 512], F32, tag="pp_sm", name=f"qkps{part}{nj}"
                        )
                for kc in range(NT_C):
                    for part in range(2):
                        for nj in range(2):
                            nc.tensor.matmul(
                                ps[part, nj][:],
                                wqk_t[:, kc * 256 + part * 128 : kc * 256 + (part + 1) * 128],
                                xT[:, kc * N + nj * 512 : kc * N + nj * 512 + 512],
                                start=(kc == 0),
                                stop=(kc == NT_C - 1),
                            )
                for part in range(2):
                    for nj in range(2):
                        nc.vector.tensor_copy(
                            qk_sb[:, part * N + nj * 512 : part * N + nj * 512 + 512],
                            ps[part, nj][:],
                        )
                return qk_sb

            def emit_qk_group(qk_sb, wqk_t, part, nj):
                """One (part, nj) quarter of a pair's qk GEMM: 6 MMs + copy."""
                ps = pp_sm.tile([128, 512], F32, tag="pp_sm")
                for kc in range(NT_C):
                    nc.tensor.matmul(
                        ps[:],
                        wqk_t[:, kc * 256 + part * 128 : kc * 256 + (part + 1) * 128],
                        xT[:, kc * N + nj * 512 : kc * N + nj * 512 + 512],
                        start=(kc == 0),
                        stop=(kc == NT_C - 1),
                    )
                nc.vector.tensor_copy(
                    qk_sb[:, part * N + nj * 512 : part * N + nj * 512 + 512], ps[:]
                )

            def emit_v_group(nb):
                """v GEMM for one n-block: v_nat[nb] = xT[:, nb].T @ wvT."""
                ps1 = pp_sm.tile([128, 512], F32, tag="pp_sm")
                ps2 = pp_sm.tile([128, 256], F32, tag="pp_sm")
                for kc in range(NT_C):
                    lhsT = xT[:, kc * N + nb * 128 : kc * N + (nb + 1) * 128]
                    nc.tensor.matmul(
                        ps1[:], lhsT, wv[:, kc * C : kc * C + 512],
                        start=(kc == 0), stop=(kc == NT_C - 1),
                    )
                    nc.tensor.matmul(
                        ps2[:], lhsT, wv[:, kc * C + 512 : kc * C + 768],
                        start=(kc == 0), stop=(kc == NT_C - 1),
                    )
                nc.vector.tensor_copy(vnat[:, nb * C : nb * C + 512], ps1[:])
                nc.vector.tensor_copy(vnat[:, nb * C + 512 : nb * C + 768], ps2[:])

            def emit_vn_copies(h, t):
                """Fill vn slot (h, t) from v_nat block t (ones col pre-set)."""
                s = (h * NT_N + t) * 65
                nc.vector.tensor_copy(
                    vn[:, s : s + 64], vnat[:, t * C + h * 64 : t * C + (h + 1) * 64]
                )

            def emit_st(qk_sb, t, ets):
                """Row-tiled concurrent ST pair for heads h0 (rows 0:64) and h1."""
                pss = []
                for h2 in range(2):
                    ps = pp_st.tile([128, N], F32, tag="pp_st")
                    pss.append(ps)
                for nj in range(2):
                    nsl = slice(nj * 512, (nj + 1) * 512)
                    for h2 in range(2):
                        rsl = slice(h2 * 64, h2 * 64 + 64)
                        nc.tensor.matmul(
                            pss[h2][:, nsl],
                            qk_sb[rsl, N + t * 128 : N + (t + 1) * 128],
                            qk_sb[rsl, nsl],
                            start=True,
                            stop=True,
                        )
                for h2 in range(2):
                    e = p_et.tile([128, N], BF16, tag="et")
                    nc.scalar.activation(e[:], pss[h2][:], EXP, scale=SCALE)
                    ets[h2].append(e)

            def emit_pv(h, nj, ets, po):
                """PV for one (head, n-half): accumulate 8 key tiles, M=65."""
                p_ = pp_sm.tile([65, 512], F32, tag="pp_sm")
                po.append(p_)
                nsl = slice(nj * 512, (nj + 1) * 512)
                for t in range(NT_N):
                    s = (h * NT_N + t) * 65
                    nc.tensor.matmul(
                        p_[:],
                        vn[:, s : s + 65],
                        ets[t][:, nsl],
                        start=(t == 0),
                        stop=(t == NT_N - 1),
                    )

            def emit_pv_copy(po, pvs):
                """Move PV psum (unnorm + den row) to SBUF bf16, freeing the
                bank (bf16 so the den bcast matmul avoids fp32 HIGH mode)."""
                for nj in range(2):
                    pv = p_pvs.tile([65, 512], BF16, tag="pvs")
                    nc.vector.tensor_copy(pv[:], po[nj][:])
                    pvs.append(pv)

            def emit_bcast(pvs, pbs):
                """K=1 PE matmul broadcasting den across 64 partitions (bf16 --
                f32r here would run fp32_mode=HIGH at ~2x cost and disable FWL
                for the following matmul)."""
                for nj in range(2):
                    pb = pp_sm.tile([64, 512], F32, tag="pp_sm")
                    nc.tensor.matmul(
                        pb[:], ones_bb[64:65, 0:64], pvs[nj][64:65, :],
                        start=True, stop=True,
                    )
                    pbs.append(pb)

            def emit_norm(h2, OT_hp, pvs, pbs):
                for nj in range(2):
                    nsl = slice(nj * 512, (nj + 1) * 512)
                    rcb = p_rcb.tile([64, 512], F32, tag="rcb")
                    nc.vector.reciprocal_approx_fast(rcb[:], pbs[nj][:])
                    nc.vector.tensor_mul(
                        OT_hp[h2 * 64 : h2 * 64 + 64, nsl], pvs[nj][0:64, :], rcb[:]
                    )

            # ---------------- PE warm-up ----------------
            # ~8 x 512-row dummy matmuls keep the PE busy through the HAM
            # SHORT window while the first DMA chunks land, so the pair-0
            # qk GEMM runs at 2.4 GHz instead of 1.2.
            ones_bb = singles.tile([128, 128], BF16, tag="ones_bb")
            nc.vector.tensor_copy(ones_bb[:], ones_f[:])
            warm_src = singles.tile([128, 512], BF16, tag="warm_src")
            nc.vector.memset(warm_src[:], 0.0)
            ps_warm = pp_sm.tile([128, 512], F32, tag="pp_sm", name="ps_warm")
            for _ in range(8):
                nc.tensor.matmul(
                    ps_warm[:], ones_bb[:], warm_src[:], start=True, stop=True
                )

            # bias broadcast tile [128, C] f32 (folded into the output copy as
            # a DVE add -- a K=1 f32r bias matmul per proj tile would run in
            # slow fp32 HIGH mode and disable FWL)
            bias_t = singles.tile([128, C], F32, tag="bias_t")
            psb1 = pp_sm.tile([128, 512], F32, tag="pp_sm", name="psb1")
            psb2 = pp_sm.tile([128, 256], F32, tag="pp_sm", name="psb2")
            nc.tensor.matmul(
                psb1[:], ones_r[0:1, 0:128], b_row[:, 0:512], start=True, stop=True
            )
            nc.tensor.matmul(
                psb2[:], ones_r[0:1, 0:128], b_row[:, 512:768], start=True, stop=True
            )
            nc.vector.tensor_copy(bias_t[:, 0:512], psb1[:])
            nc.vector.tensor_copy(bias_t[:, 512:768], psb2[:])

            # ---------------- pair 0 qk GEMM (DMA-pipelined) ----------------
            qk_cur = emit_qk_first(wqk_cur)

            # ---------------- steady-state pair loop ----------------
            ets_prev = None   # [h2] -> list of 8 et tiles, previous pair
            hp_prev = None
            OT = [
                p_OT.tile([128, N], BF16, tag="OT", name=f"OT{j}")
                for j in range(NPAIR)
            ]

            for hp in range(NPAIR):
                if hp + 1 < NPAIR:
                    wqk_next = dma_wqk(hp + 1)
                    qk_next = p_qk.tile([128, 2 * N], BF16, tag="qk", name=f"qk{hp+1}")
                ets = [[], []]
                # filler state for PV(hp-1) + norm
                po_h = [[], []]
                pvs_h = [[], []]
                pbs_h = [[], []]
                if hp_prev is not None:
                    OT_prev = OT[hp_prev]

                for t in range(NT_N):
                    emit_st(qk_cur, t, ets)
                    if hp == 0:
                        # pair 0 fillers: v GEMM + vn assembly + qk GEMM pair 1
                        emit_v_group(t)
                        for h in (0, 1):  # vn slots for pair 0's heads
                            emit_vn_copies(h, t)
                        if t in (2, 3, 6, 7):
                            part, nj = {2: (0, 0), 3: (0, 1), 6: (1, 0), 7: (1, 1)}[t]
                            emit_qk_group(qk_next, wqk_next, part, nj)
                    else:
                        h_lo = 2 * hp_prev
                        if t == 0:
                            emit_pv(h_lo, 0, ets_prev[0], po_h[0])
                        elif t == 1:
                            emit_pv(h_lo, 1, ets_prev[0], po_h[0])
                        elif t == 2:
                            emit_pv_copy(po_h[0], pvs_h[0])
                            emit_bcast(pvs_h[0], pbs_h[0])
                            if hp + 1 < NPAIR:
                                emit_qk_group(qk_next, wqk_next, 0, 0)
                        elif t == 3:
                            emit_norm(0, OT_prev, pvs_h[0], pbs_h[0])
                            if hp + 1 < NPAIR:
                                emit_qk_group(qk_next, wqk_next, 0, 1)
                        elif t == 4:
                            emit_pv(h_lo + 1, 0, ets_prev[1], po_h[1])
                        elif t == 5:
                            emit_pv(h_lo + 1, 1, ets_prev[1], po_h[1])
                        elif t == 6:
                            emit_pv_copy(po_h[1], pvs_h[1])
                            emit_bcast(pvs_h[1], pbs_h[1])
                            if hp + 1 < NPAIR:
                                emit_qk_group(qk_next, wqk_next, 1, 0)
                        elif t == 7:
                            emit_norm(1, OT_prev, pvs_h[1], pbs_h[1])
                            if hp + 1 < NPAIR:
                                emit_qk_group(qk_next, wqk_next, 1, 1)
                        # vn slots for this pair's heads (needed by PV at hp+1)
                        for h in (2 * hp, 2 * hp + 1):
                            emit_vn_copies(h, t)

                ets_prev = ets
                hp_prev = hp
                if hp + 1 < NPAIR:
                    qk_cur = qk_next
                    wqk_cur = wqk_next

            # ---------------- tail: PV + norm of last pair, interleaved with
            # proj partials for nb 0/1 so the PE never idles through the
            # final exp drain (idle >3.4us would re-throttle the HAM).
            def emit_proj_partial(ps, nb, jmax):
                for osl in (slice(0, 512), slice(512, 768)):
                    for j in range(jmax):
                        nc.tensor.matmul(
                            ps[:, osl],
                            OT[j][:, nb * 128 : (nb + 1) * 128],
                            wp[:, j * C + osl.start : j * C + osl.stop],
                            start=(j == 0),
                            stop=False,
                        )

            def emit_proj_finish(ps, nb, jmin):
                for osl in (slice(0, 512), slice(512, 768)):
                    for j in range(jmin, NPAIR):
                        nc.tensor.matmul(
                            ps[:, osl],
                            OT[j][:, nb * 128 : (nb + 1) * 128],
                            wp[:, j * C + osl.start : j * C + osl.stop],
                            start=(jmin == 0 and j == 0),
                            stop=(j == NPAIR - 1),
                        )
                osb = p_osb.tile([128, C], F32, tag="osb", name=f"osb{nb}")
                nc.vector.tensor_add(osb[:], ps[:, 0:C], bias_t[:])
                nc.sync.dma_start(out_d[nb * 128 : (nb + 1) * 128, :], osb[:])

            # PV(5, h0) + its norm chain run first so OT[5][0:64] is done
            # early; PV(5, h1) + proj partials keep the PE busy while the h0
            # and h1 norm chains (DVE) drain.
            h_lo = 2 * hp_prev
            OT_prev = OT[hp_prev]
            po_h = [[], []]
            pvs_h = [[], []]
            pbs_h = [[], []]

            ps_nb0 = pp_st.tile([128, N], F32, tag="pp_st", name="ps_nb0")
            emit_proj_partial(ps_nb0, 0, NPAIR - 1)
            emit_pv(h_lo, 0, ets_prev[0], po_h[0])
            emit_pv(h_lo, 1, ets_prev[0], po_h[0])
            emit_pv_copy(po_h[0], pvs_h[0])
            emit_bcast(pvs_h[0], pbs_h[0])
            emit_norm(0, OT_prev, pvs_h[0], pbs_h[0])
            ps_nb1 = pp_st.tile([128, N], F32, tag="pp_st", name="ps_nb1")
            emit_proj_partial(ps_nb1, 1, NPAIR - 1)
            emit_pv(h_lo + 1, 0, ets_prev[1], po_h[1])
            emit_pv(h_lo + 1, 1, ets_prev[1], po_h[1])
            emit_pv_copy(po_h[1], pvs_h[1])
            emit_bcast(pvs_h[1], pbs_h[1])
            emit_norm(1, OT_prev, pvs_h[1], pbs_h[1])

            # ---------------- proj ----------------
            emit_proj_finish(ps_nb0, 0, NPAIR - 1)
            emit_proj_finish(ps_nb1, 1, NPAIR - 1)
            for nb in range(2, NT_N):
                ps = pp_st.tile([128, N], F32, tag="pp_st", name=f"ps_nb{nb}")
                emit_proj_finish(ps, nb, 0)

    nc.compile()
    return nc


_NC_CACHE = None


def _prep_inputs(x, w_qkv, w_proj, b_proj):
    x = np.asarray(x, dtype=np.float32)
    w_qkv = np.asarray(w_qkv, dtype=np.float32)
    w_proj = np.asarray(w_proj, dtype=np.float32)
    b_row = np.ascontiguousarray(
        np.asarray(b_proj, dtype=np.float32).reshape(1, C)
    )

    # per-pair packed [768, 256] blocks: cols 0:128 = q rows of the pair
    # transposed, cols 128:256 = k rows of the pair transposed
    wqk_blocks = []
    for hp in range(NPAIR):
        qb = w_qkv[hp * 128 : (hp + 1) * 128, :]          # [128, 768]
        kb = w_qkv[C + hp * 128 : C + (hp + 1) * 128, :]  # [128, 768]
        wqk_blocks.append(np.concatenate([qb.T, kb.T], axis=1))  # [768, 256]
    wqk = np.ascontiguousarray(np.concatenate(wqk_blocks, axis=0)).astype(BF)
    wv = np.ascontiguousarray(w_qkv[2 * C :, :].T).astype(BF)   # [768, 768]
    wp = np.ascontiguousarray(w_proj.T).astype(BF)              # [768, 768]
    xTs = [np.ascontiguousarray(x[b].T).astype(BF) for b in range(NCORES)]
    return xTs, wqk, wv, wp, b_row


def kernel(x, w_qkv, w_proj, b_proj):
    global _NC_CACHE
    if _NC_CACHE is None:
        _NC_CACHE = build_bass()
    nc = _NC_CACHE

    xTs, wqk, wv, wp, b_row = _prep_inputs(x, w_qkv, w_proj, b_proj)
    in_maps = [
        {"xT": xTs[b], "wqk": wqk, "wv": wv, "wp": wp, "b_proj": b_row}
        for b in range(NCORES)
    ]
    res = run_bass_kernel_spmd(nc, in_maps, list(range(NCORES)))
    return np.stack([res.results[b]["out"] for b in range(NCORES)], axis=0)


# revision 36
# speedup vs baseline: 1.0770x; 1.0043x over previous
"""Trainium2 Bass kernel for multi-head self-attention (B=8, N=1024, C=768, H=12).

Sharding: data-parallel over batch -- one batch element per NeuronCore (8 cores).

Key design points (v2, rewritten from the f32r baseline):
  * All operand transposes are done on the HOST (numpy) -- the device kernel
    contains ZERO PE-transpose instructions.  PE transposes don't count as
    PE-busy for the HAM clock gate, so the baseline's 288 transposes both
    cost ~80us of PE slices and kept re-throttling the PE to 1.2 GHz.
  * Everything is bf16 (1.0 cycles/row, FWL-eligible weight loads, half DMA).
  * v is computed in NATURAL [n, c] layout via an xT-stationary GEMM
    (out[n,vc] = xT[k,n].T @ wvT[k,vc]) so the per-head [keys, 64|1] PV
    stationary tiles are built with cheap DVE copies instead of PE transposes.
  * ST score matmuls have K=64: the two heads of a pair sit on SBUF
    partitions 0:64 / 64:128, so their matmuls land on disjoint PE row
    groups (tile_position (0,0) vs (64,0)) and execute CONCURRENTLY when
    issued back-to-back (~2x effective ST throughput).
  * Per head pair the PE work (qk GEMM for next pair + ST + PV + norm
    broadcasts ~16.2us) matches the ACT exp work (16 x [128,1024] exp =
    16.4us), so both engines stay ~100% busy and the HAM stays at K=8/8.

Per-core dataflow:
  qkT  [256, N] per pair = wqkT_pair.T @ xT   (bf16, stationary = w slices)
  v    [N, C]  = xT.T @ wvT                    (bf16, natural layout)
  per head h:  ST[m,n] = k_h @ q_h^T           (bf16, K=64, row-tiled pairs)
               ET = exp(0.125*ST) -> bf16      (one ACT op per [128,1024])
               PV: [v_h | 1].T @ ET -> [65, n] unnormalized + denominator
               OT[d,n] = PV[0:64] * bcast(den)^-1   (K=1 PE bcast + DVE)
  out [N, C] = OT.T @ wprojT + b_proj          (bf16, bias as K=1 f32r matmul)
"""

import numpy as np
import ml_dtypes

import concourse.bass as bass
import concourse.tile as tile
from concourse import bacc
from concourse import mybir
from concourse.bass_utils import run_bass_kernel_spmd

N = 1024
C = 768
H = 12
D = 64
NCORES = 8
SCALE = D**-0.5

F32 = mybir.dt.float32
F32R = mybir.dt.float32r
BF16 = mybir.dt.bfloat16
F8 = mybir.dt.float8e4
EXP = mybir.ActivationFunctionType.Exp

FP8_PV = False     # exp->fp8e4 + DoubleRow PV: rel err 2.1e-2, over the 2e-2
                   # gate, and only ~2us faster than bf16 -- keep bf16
ET_DT = F8 if FP8_PV else BF16
VN_STRIDE = 80 if FP8_PV else 65  # DoubleRow weight AP needs step % 16 == 0

NT_N = N // 128  # 8 n-blocks / key tiles
NT_C = C // 128  # 6 k-chunks
NPAIR = H // 2   # 6 head pairs

BF = ml_dtypes.bfloat16


def build_bass():
    nc = bacc.Bacc("TRN2", target_bir_lowering=False, debug=False, num_devices=NCORES)

    # host-pretransposed inputs
    xT_d = nc.dram_tensor("xT", [C, N], BF16, kind="ExternalInput").ap()
    # per-pair packed [q_pair(128) | k_pair(128)] columns: [NPAIR*768, 256]
    wqk_d = nc.dram_tensor("wqk", [NPAIR * C, 256], BF16, kind="ExternalInput").ap()
    wv_d = nc.dram_tensor("wv", [C, C], BF16, kind="ExternalInput").ap()  # w_v^T
    wp_d = nc.dram_tensor("wp", [C, C], BF16, kind="ExternalInput").ap()  # w_proj^T
    b_d = nc.dram_tensor("b_proj", [1, C], F32R, kind="ExternalInput").ap()
    out_d = nc.dram_tensor("out", [N, C], F32, kind="ExternalOutput").ap()

    with tile.TileContext(nc) as tc:
        with (
            tc.tile_pool(name="singles", bufs=1) as singles,
            tc.tile_pool(name="wqk", bufs=2) as p_wqk,
            tc.tile_pool(name="qk", bufs=2) as p_qk,
            tc.tile_pool(name="et", bufs=(16 if FP8_PV else 32)) as p_et,
            tc.tile_pool(name="OT", bufs=NPAIR) as p_OT,
            tc.tile_pool(name="pvs", bufs=4) as p_pvs,
            tc.tile_pool(name="rcb", bufs=4) as p_rcb,
            tc.tile_pool(name="rcbb", bufs=4) as p_rcbb,
            tc.tile_pool(name="osb", bufs=2) as p_osb,
            # PSUM: 8 banks total
            tc.tile_pool(name="pp_st", bufs=2, space="PSUM") as pp_st,  # 2x2 banks
            tc.tile_pool(name="pp_sm", bufs=4, space="PSUM") as pp_sm,  # 4x1 bank
        ):
            # ---------------- setup ----------------
            ones_f = singles.tile([128, 128], F32, tag="ones_f")
            nc.vector.memset(ones_f[:], 1.0)
            ones_r = singles.tile([128, 128], F32R, tag="ones_r")
            nc.vector.tensor_copy(ones_r[:], ones_f[:])
            ones_b = singles.tile([128, 1], BF16, tag="ones_b")
            nc.vector.tensor_copy(ones_b[:], ones_f[:, 0:1])
            b_row = singles.tile([1, C], F32R, tag="b_row")
            nc.sync.dma_start(b_row[:], b_d)

            # persistent SBUF planes
            xT = singles.tile([128, NT_C * N], BF16, tag="xT")      # [k, n] chunks
            wv = singles.tile([128, NT_C * C], BF16, tag="wv")      # [k, vc] chunks
            wp = singles.tile([128, NT_C * C], BF16, tag="wp")      # [cj, oc] chunks
            vnat = singles.tile([128, NT_N * C], BF16, tag="vnat")  # [n, vc] blocks
            # per (h, t) PV stationary slots [keys, v(64) | ones]
            vn = singles.tile([128, H * NT_N * VN_STRIDE], ET_DT, tag="vn")

            # ones column of every vn slot
            ones_e = singles.tile([128, 1], ET_DT, tag="ones_e")
            nc.vector.tensor_copy(ones_e[:], ones_f[:, 0:1])
            ones_cols = bass.AP(
                tensor=vn.tensor,
                offset=vn.offset + 64,
                ap=[vn.ap[0], [VN_STRIDE, H * NT_N], [1, 1]],
            )
            ones_rep = bass.AP(
                tensor=ones_e.tensor,
                offset=ones_e.offset,
                ap=[ones_e.ap[0], [0, H * NT_N], [1, 1]],
            )
            nc.vector.tensor_copy(ones_cols, ones_rep)

            # ---------------- input DMAs ----------------
            def dma_wqk(hp):
                t = p_wqk.tile([128, NT_C * 256], BF16, tag="wqk", name=f"wqk{hp}")
                for kc in range(NT_C):
                    nc.sync.dma_start(
                        t[:, kc * 256 : (kc + 1) * 256],
                        wqk_d[hp * C + kc * 128 : hp * C + (kc + 1) * 128, :],
                    )
                return t

            # interleave wqk0/xT chunk loads so the pair-0 qk GEMM can start
            # on chunk 0 ~1.2us in
            wqk_cur = p_wqk.tile([128, NT_C * 256], BF16, tag="wqk", name="wqk0")
            for kc in range(NT_C):
                nc.sync.dma_start(
                    wqk_cur[:, kc * 256 : (kc + 1) * 256],
                    wqk_d[kc * 128 : (kc + 1) * 128, :],
                )
                nc.sync.dma_start(
                    xT[:, kc * N : (kc + 1) * N], xT_d[kc * 128 : (kc + 1) * 128, :]
                )
            for kc in range(NT_C):
                nc.sync.dma_start(
                    wv[:, kc * C : (kc + 1) * C], wv_d[kc * 128 : (kc + 1) * 128, :]
                )
            for kc in range(NT_C):
                nc.sync.dma_start(
                    wp[:, kc * C : (kc + 1) * C], wp_d[kc * 128 : (kc + 1) * 128, :]
                )

            # ---------------- emission helpers ----------------
            def emit_qk_first(wqk_t):
                """Pair-0 qk GEMM, k-chunk outer so PE starts as DMA lands."""
                qk_sb = p_qk.tile([128, 2 * N], BF16, tag="qk", name="qk0")
                ps = {}
                for part in range(2):
                    for nj in range(2):
                        ps[part, nj] = pp_sm.tile(
                            [128, 512], F32, tag="pp_sm", name=f"qkps{part}{nj}"
                        )
                for kc in range(NT_C):
                    for part in range(2):
                        for nj in range(2):
                            nc.tensor.matmul(
                                ps[part, nj][:],
                                wqk_t[:, kc * 256 + part * 128 : kc * 256 + (part + 1) * 128],
                                xT[:, kc * N + nj * 512 : kc * N + nj * 512 + 512],
                                start=(kc == 0),
                                stop=(kc == NT_C - 1),
                            )
                for part in range(2):
                    for nj in range(2):
                        nc.vector.tensor_copy(
                            qk_sb[:, part * N + nj * 512 : part * N + nj * 512 + 512],
                            ps[part, nj][:],
                        )
                return qk_sb

            def emit_qk_group(qk_sb, wqk_t, part, nj):
                """One (part, nj) quarter of a pair's qk GEMM: 6 MMs + copy."""
                ps = pp_sm.tile([128, 512], F32, tag="pp_sm")
                for kc in range(NT_C):
                    nc.tensor.matmul(
                        ps[:],
                        wqk_t[:, kc * 256 + part * 128 : kc * 256 + (part + 1) * 128],
                        xT[:, kc * N + nj * 512 : kc * N + nj * 512 + 512],
                        start=(kc == 0),
                        stop=(kc == NT_C - 1),
                    )
                nc.vector.tensor_copy(
                    qk_sb[:, part * N + nj * 512 : part * N + nj * 512 + 512], ps[:]
                )

            def emit_v_group(nb):
                """v GEMM for one n-block: v_nat[nb] = xT[:, nb].T @ wvT."""
                ps1 = pp_sm.tile([128, 512], F32, tag="pp_sm")
                ps2 = pp_sm.tile([128, 256], F32, tag="pp_sm")
                for kc in range(NT_C):
                    lhsT = xT[:, kc * N + nb * 128 : kc * N + (nb + 1) * 128]
                    nc.tensor.matmul(
                        ps1[:], lhsT, wv[:, kc * C : kc * C + 512],
                        start=(kc == 0), stop=(kc == NT_C - 1),
                    )
                    nc.tensor.matmul(
                        ps2[:], lhsT, wv[:, kc * C + 512 : kc * C + 768],
                        start=(kc == 0), stop=(kc == NT_C - 1),
                    )
                nc.vector.tensor_copy(vnat[:, nb * C : nb * C + 512], ps1[:])
                nc.vector.tensor_copy(vnat[:, nb * C + 512 : nb * C + 768], ps2[:])

            def emit_vn_copies(h, t):
                """Fill vn slot (h, t) from v_nat block t (ones col pre-set)."""
                s = (h * NT_N + t) * VN_STRIDE
                nc.vector.tensor_copy(
                    vn[:, s : s + 64], vnat[:, t * C + h * 64 : t * C + (h + 1) * 64]
                )

            def emit_st(qk_sb, t, ets):
                """Row-tiled concurrent ST pair for heads h0 (rows 0:64) and h1."""
                pss = []
                for h2 in range(2):
                    ps = pp_st.tile([128, N], F32, tag="pp_st")
                    pss.append(ps)
                for nj in range(2):
                    nsl = slice(nj * 512, (nj + 1) * 512)
                    for h2 in range(2):
                        rsl = slice(h2 * 64, h2 * 64 + 64)
                        nc.tensor.matmul(
                            pss[h2][:, nsl],
                            qk_sb[rsl, N + t * 128 : N + (t + 1) * 128],
                            qk_sb[rsl, nsl],
                            start=True,
                            stop=True,
                        )
                for h2 in range(2):
                    if FP8_PV:
                        # pack key-tile pairs (t, t+1) into one [128, 2N] tile
                        # so the DoubleRow rhs AP can span both
                        if t % 2 == 0:
                            e = p_et.tile([128, 2 * N], ET_DT, tag="et")
                            ets[h2].append(e)
                        dst = ets[h2][-1][:, (t % 2) * N : (t % 2 + 1) * N]
                    else:
                        e = p_et.tile([128, N], ET_DT, tag="et")
                        ets[h2].append(e)
                        dst = e[:]
                    nc.scalar.activation(dst, pss[h2][:], EXP, scale=SCALE)

            def emit_pv(h, nj, ets, po):
                """PV for one (head, n-half): accumulate 8 key tiles, M=65."""
                p_ = pp_sm.tile([65, 512], F32, tag="pp_sm")
                po.append(p_)
                if FP8_PV:
                    # fp8 DoubleRow: two 128-key subtiles per matmul
                    for tp in range(NT_N // 2):
                        s = (h * NT_N + 2 * tp) * VN_STRIDE
                        lhsT = bass.AP(
                            tensor=vn.tensor,
                            offset=vn.offset + s,
                            ap=[vn.ap[0], [VN_STRIDE, 2], [1, 65]],
                        )
                        et_t = ets[tp]
                        rhs = bass.AP(
                            tensor=et_t.tensor,
                            offset=et_t.offset + nj * 512,
                            ap=[et_t.ap[0], [N, 2], [1, 512]],
                        )
                        nc.tensor.matmul(
                            p_[:],
                            lhsT,
                            rhs,
                            start=(tp == 0),
                            stop=(tp == NT_N // 2 - 1),
                            perf_mode=mybir.MatmulPerfMode.DoubleRow,
                        )
                    return
                nsl = slice(nj * 512, (nj + 1) * 512)
                for t in range(NT_N):
                    s = (h * NT_N + t) * VN_STRIDE
                    nc.tensor.matmul(
                        p_[:],
                        vn[:, s : s + 65],
                        ets[t][:, nsl],
                        start=(t == 0),
                        stop=(t == NT_N - 1),
                    )

            def emit_pv_copy(po, pvs):
                """Move PV psum (unnorm + den row) to SBUF bf16, freeing the
                bank (bf16 so the den bcast matmul avoids fp32 HIGH mode)."""
                for nj in range(2):
                    pv = p_pvs.tile([65, 512], BF16, tag="pvs")
                    nc.vector.tensor_copy(pv[:], po[nj][:])
                    pvs.append(pv)

            def emit_bcast(pvs, pbs):
                """K=1 PE matmul broadcasting den across 64 partitions (bf16 --
                f32r here would run fp32_mode=HIGH at ~2x cost and disable FWL
                for the following matmul)."""
                for nj in range(2):
                    pb = pp_sm.tile([64, 512], F32, tag="pp_sm")
                    nc.tensor.matmul(
                        pb[:], ones_bb[64:65, 0:64], pvs[nj][64:65, :],
                        start=True, stop=True,
                    )
                    pbs.append(pb)

            def emit_norm(h2, OT_hp, pvs, pbs):
                for nj in range(2):
                    nsl = slice(nj * 512, (nj + 1) * 512)
                    rcb = p_rcb.tile([64, 512], F32, tag="rcb")
                    nc.vector.reciprocal_approx_fast(rcb[:], pbs[nj][:])
                    nc.vector.tensor_mul(
                        OT_hp[h2 * 64 : h2 * 64 + 64, nsl], pvs[nj][0:64, :], rcb[:]
                    )

            # ---------------- PE warm-up ----------------
            # ~8 x 512-row dummy matmuls keep the PE busy through the HAM
            # SHORT window while the first DMA chunks land, so the pair-0
            # qk GEMM runs at 2.4 GHz instead of 1.2.
            ones_bb = singles.tile([128, 128], BF16, tag="ones_bb")
            nc.vector.tensor_copy(ones_bb[:], ones_f[:])
            warm_src = singles.tile([128, 512], BF16, tag="warm_src")
            nc.vector.memset(warm_src[:], 0.0)
            ps_warm = pp_sm.tile([128, 512], F32, tag="pp_sm", name="ps_warm")
            for _ in range(8):
                nc.tensor.matmul(
                    ps_warm[:], ones_bb[:], warm_src[:], start=True, stop=True
                )

            # bias broadcast tile [128, C] f32 (folded into the output copy as
            # a DVE add -- a K=1 f32r bias matmul per proj tile would run in
            # slow fp32 HIGH mode and disable FWL)
            bias_t = singles.tile([128, C], F32, tag="bias_t")
            psb1 = pp_sm.tile([128, 512], F32, tag="pp_sm", name="psb1")
            psb2 = pp_sm.tile([128, 256], F32, tag="pp_sm", name="psb2")
            nc.tensor.matmul(
                psb1[:], ones_r[0:1, 0:128], b_row[:, 0:512], start=True, stop=True
            )
            nc.tensor.matmul(
                psb2[:], ones_r[0:1, 0:128], b_row[:, 512:768], start=True, stop=True
            )
            nc.vector.tensor_copy(bias_t[:, 0:512], psb1[:])
            nc.vector.tensor_copy(bias_t[:, 512:768], psb2[:])

            # ---------------- pair 0 qk GEMM (DMA-pipelined) ----------------
            qk_cur = emit_qk_first(wqk_cur)

            # ---------------- steady-state pair loop ----------------
            ets_prev = None   # [h2] -> list of 8 et tiles, previous pair
            hp_prev = None
            OT = [
                p_OT.tile([128, N], BF16, tag="OT", name=f"OT{j}")
                for j in range(NPAIR)
            ]

            for hp in range(NPAIR):
                if hp + 1 < NPAIR:
                    wqk_next = dma_wqk(hp + 1)
                    qk_next = p_qk.tile([128, 2 * N], BF16, tag="qk", name=f"qk{hp+1}")
                ets = [[], []]
                # filler state for PV(hp-1) + norm
                po_h = [[], []]
                pvs_h = [[], []]
                pbs_h = [[], []]
                if hp_prev is not None:
                    OT_prev = OT[hp_prev]

                for t in range(NT_N):
                    emit_st(qk_cur, t, ets)
                    if hp == 0:
                        # pair 0 fillers: v GEMM + vn assembly + qk GEMM pair 1
                        emit_v_group(t)
                        for h in (0, 1):  # vn slots for pair 0's heads
                            emit_vn_copies(h, t)
                        if t in (2, 3, 6, 7):
                            part, nj = {2: (0, 0), 3: (0, 1), 6: (1, 0), 7: (1, 1)}[t]
                            emit_qk_group(qk_next, wqk_next, part, nj)
                    else:
                        h_lo = 2 * hp_prev
                        if t == 0:
                            emit_pv(h_lo, 0, ets_prev[0], po_h[0])
                        elif t == 1:
                            emit_pv(h_lo, 1, ets_prev[0], po_h[0])
                        elif t == 2:
                            emit_pv_copy(po_h[0], pvs_h[0])
                            emit_bcast(pvs_h[0], pbs_h[0])
                            if hp + 1 < NPAIR:
                                emit_qk_group(qk_next, wqk_next, 0, 0)
                        elif t == 3:
                            emit_norm(0, OT_prev, pvs_h[0], pbs_h[0])
                            if hp + 1 < NPAIR:
                                emit_qk_group(qk_next, wqk_next, 0, 1)
                        elif t == 4:
                            emit_pv(h_lo + 1, 0, ets_prev[1], po_h[1])
                        elif t == 5:
                            emit_pv(h_lo + 1, 1, ets_prev[1], po_h[1])
                        elif t == 6:
                            emit_pv_copy(po_h[1], pvs_h[1])
                            emit_bcast(pvs_h[1], pbs_h[1])
                            if hp + 1 < NPAIR:
                                emit_qk_group(qk_next, wqk_next, 1, 0)
                        elif t == 7:
                            emit_norm(1, OT_prev, pvs_h[1], pbs_h[1])
                            if hp + 1 < NPAIR:
                                emit_qk_group(qk_next, wqk_next, 1, 1)
                        # vn slots for this pair's heads (needed by PV at hp+1)
                        for h in (2 * hp, 2 * hp + 1):
                            emit_vn_copies(h, t)

                ets_prev = ets
                hp_prev = hp
                if hp + 1 < NPAIR:
                    qk_cur = qk_next
                    wqk_cur = wqk_next

            # ---------------- tail: PV + norm of last pair, interleaved with
            # proj partials for nb 0/1 so the PE never idles through the
            # final exp drain (idle >3.4us would re-throttle the HAM).
            def emit_proj_partial(ps, nb, jmax):
                for osl in (slice(0, 512), slice(512, 768)):
                    for j in range(jmax):
                        nc.tensor.matmul(
                            ps[:, osl],
                            OT[j][:, nb * 128 : (nb + 1) * 128],
                            wp[:, j * C + osl.start : j * C + osl.stop],
                            start=(j == 0),
                            stop=False,
                        )

            def emit_proj_finish(ps, nb, jmin):
                for osl in (slice(0, 512), slice(512, 768)):
                    for j in range(jmin, NPAIR):
                        nc.tensor.matmul(
                            ps[:, osl],
                            OT[j][:, nb * 128 : (nb + 1) * 128],
                            wp[:, j * C + osl.start : j * C + osl.stop],
                            start=(jmin == 0 and j == 0),
                            stop=(j == NPAIR - 1),
                        )
                osb = p_osb.tile([128, C], F32, tag="osb", name=f"osb{nb}")
                nc.vector.tensor_add(osb[:], ps[:, 0:C], bias_t[:])
                nc.sync.dma_start(out_d[nb * 128 : (nb + 1) * 128, :], osb[:])

            # PV(5, h0)'s norm chain (DVE) drains under PV(5, h1); PV(5, h1)'s
            # drains under the proj partial for nb1, so the PE reaches
            # proj_finish with OT[5] already complete.
            h_lo = 2 * hp_prev
            OT_prev = OT[hp_prev]
            po_h = [[], []]
            pvs_h = [[], []]
            pbs_h = [[], []]

            ps_nb0 = pp_st.tile([128, N], F32, tag="pp_st", name="ps_nb0")
            emit_proj_partial(ps_nb0, 0, NPAIR - 1)
            emit_pv(h_lo, 0, ets_prev[0], po_h[0])
            emit_pv(h_lo, 1, ets_prev[0], po_h[0])
            emit_pv_copy(po_h[0], pvs_h[0])
            emit_bcast(pvs_h[0], pbs_h[0])
            emit_norm(0, OT_prev, pvs_h[0], pbs_h[0])
            emit_pv(h_lo + 1, 0, ets_prev[1], po_h[1])
            emit_pv(h_lo + 1, 1, ets_prev[1], po_h[1])
            emit_pv_copy(po_h[1], pvs_h[1])
            emit_bcast(pvs_h[1], pbs_h[1])
            emit_norm(1, OT_prev, pvs_h[1], pbs_h[1])
            ps_nb1 = pp_st.tile([128, N], F32, tag="pp_st", name="ps_nb1")
            emit_proj_partial(ps_nb1, 1, NPAIR - 1)

            # ---------------- proj ----------------
            emit_proj_finish(ps_nb0, 0, NPAIR - 1)
            emit_proj_finish(ps_nb1, 1, NPAIR - 1)
            for nb in range(2, NT_N):
                ps = pp_st.tile([128, N], F32, tag="pp_st", name=f"ps_nb{nb}")
                emit_proj_finish(ps, nb, 0)

    nc.compile()
    return nc


_NC_CACHE = None


def _prep_inputs(x, w_qkv, w_proj, b_proj):
    x = np.asarray(x, dtype=np.float32)
    w_qkv = np.asarray(w_qkv, dtype=np.float32)
    w_proj = np.asarray(w_proj, dtype=np.float32)
    b_row = np.ascontiguousarray(
        np.asarray(b_proj, dtype=np.float32).reshape(1, C)
    )

    # per-pair packed [768, 256] blocks: cols 0:128 = q rows of the pair
    # transposed, cols 128:256 = k rows of the pair transposed
    wqk_blocks = []
    for hp in range(NPAIR):
        qb = w_qkv[hp * 128 : (hp + 1) * 128, :]          # [128, 768]
        kb = w_qkv[C + hp * 128 : C + (hp + 1) * 128, :]  # [128, 768]
        wqk_blocks.append(np.concatenate([qb.T, kb.T], axis=1))  # [768, 256]
    wqk = np.ascontiguousarray(np.concatenate(wqk_blocks, axis=0)).astype(BF)
    wv = np.ascontiguousarray(w_qkv[2 * C :, :].T).astype(BF)   # [768, 768]
    wp = np.ascontiguousarray(w_proj.T).astype(BF)              # [768, 768]
    xTs = [np.ascontiguousarray(x[b].T).astype(BF) for b in range(NCORES)]
    return xTs, wqk, wv, wp, b_row


def kernel(x, w_qkv, w_proj, b_proj):
    global _NC_CACHE
    if _NC_CACHE is None:
        _NC_CACHE = build_bass()
    nc = _NC_CACHE

    xTs, wqk, wv, wp, b_row = _prep_inputs(x, w_qkv, w_proj, b_proj)
    in_maps = [
        {"xT": xTs[b], "wqk": wqk, "wv": wv, "wp": wp, "b_proj": b_row}
        for b in range(NCORES)
    ]
    res = run_bass_kernel_spmd(nc, in_maps, list(range(NCORES)))
    return np.stack([res.results[b]["out"] for b in range(NCORES)], axis=0)
